# revision 12
# baseline (speedup 1.0000x reference)
"""Trainium2 Bass kernel for quantized (BitNet-style) multi-head attention.

Model (per batch element): bitlinear(qkv) -> 6-head softmax attention -> bitlinear(proj)
with B=8, N=2048, C=384, H=6, D=64.

Sharding: pure data parallel — one batch element per NeuronCore (8 cores),
weights replicated, no collectives.

Device algorithm highlights:
  * act/weight quantization reproduced bit-exactly (exact 128/amax divide +
    magic-number round-to-nearest-even), so the integer matmuls run on the PE
    in fp16 exactly (|ints| <= 128, ternary weights; fp32 PSUM accumulate).
  * attention computed in S^T layout: S^T[m,n] = sum_d k^T[d,m] q^T[d,n].
    D=64 -> two concurrent row-tiled matmuls (tile_position (0,0)/(64,0))
    using duplicated q/k partition halves to keep the full PE array busy.
  * softmax skips the max-subtraction (scores are bounded ~|0.5|) and gets the
    row-sums for free from an appended ones-column in V during attn @ V.
  * exp runs on ScalarE straight out of PSUM at N=2048 per instruction.
"""

import sys
import os

sys.path.insert(0, "/opt/trn_rl_repo")

import numpy as np

import concourse.bass as bass
import concourse.mybir as mybir
import concourse.tile as tile
import concourse.bacc as bacc
from concourse import bass_utils
from concourse.bass_isa import ReduceOp
from concourse.masks import make_identity

F32 = mybir.dt.float32
F16 = mybir.dt.float16
AF = mybir.ActivationFunctionType
ALU = mybir.AluOpType

B, N, C = 8, 2048, 384
H, D = 6, 64
O3 = 3 * C  # 1152
NT = N // 128   # 16 token tiles
CB = C // 128   # 3 contraction blocks
OT = O3 // 128  # 9 qkv output row tiles
MAGIC = float(1.5 * 2**23)  # fp32 round-to-nearest-even via add/sub


def build_program():
    nc = bacc.Bacc("TRN2", target_bir_lowering=False, debug=False, num_devices=8)

    x_d = nc.dram_tensor("x", [N, C], F32, kind="ExternalInput")
    w1_d = nc.dram_tensor("qkv_w", [O3, C], F32, kind="ExternalInput")
    b1_d = nc.dram_tensor("qkv_b", [O3], F32, kind="ExternalInput")
    w2_d = nc.dram_tensor("proj_w", [C, C], F32, kind="ExternalInput")
    b2_d = nc.dram_tensor("proj_b", [C], F32, kind="ExternalInput")
    y_d = nc.dram_tensor("y", [N, C], F32, kind="ExternalOutput")

    with tile.TileContext(nc) as tc:
        _body(nc, tc, x_d, w1_d, b1_d, w2_d, b2_d, y_d)
    nc.compile()
    return nc


def _body(nc, tc, x_d, w1_d, b1_d, w2_d, b2_d, y_d):
    from contextlib import ExitStack

    ctx = ExitStack()
    with ctx:
        const = ctx.enter_context(tc.tile_pool(name="const", bufs=1))
        # identities for PE transposes
        id16 = const.tile([128, 128], F16)
        make_identity(nc, id16[:])
        id32 = const.tile([128, 128], F32)
        make_identity(nc, id32[:])
        c1 = const.tile([128, 1], F32)
        nc.gpsimd.memset(c1[:], 1.0)
        c128 = const.tile([128, 1], F32)
        nc.gpsimd.memset(c128[:], 128.0)

        # ---------------- weights preamble ----------------
        t1T = [const.tile([128, O3], F16, tag=f"t1T{c}", name=f"t1T{c}") for c in range(CB)]
        t2T = [const.tile([128, C], F16, tag=f"t2T{c}", name=f"t2T{c}") for c in range(CB)]
        b1col = [const.tile([128, 1], F32, tag=f"b1c{o}", name=f"b1c{o}") for o in range(6)]
        bqcol = [const.tile([128, 1], F32, tag=f"bqc{o}", name=f"bqc{o}") for o in range(3)]
        bv_b = const.tile([128, C], F32)
        b2_b = const.tile([128, C], F32)
        mc1_128 = const.tile([128, 1], F32)
        mc1_1024 = const.tile([128, 1], F32)
        mc2_128 = const.tile([128, 1], F32)

        with (
            tc.tile_pool(name="wstage", bufs=1) as ws,
            tc.tile_pool(name="ps_tr", bufs=2, space="PSUM") as ps_tr,
        ):
            def quant_weights(w_dram, n_tiles, nelem, tag):
                """Load fp32 weight tiles, return (list of fp16 ternary tiles, mc tile)
                where mc = max(mean|w|, 1e-5) broadcast to [128,1] (equals 1/scale)."""
                wt = []
                cols = []
                for o in range(n_tiles):
                    w = ws.tile([128, C], F32, tag=f"w{tag}{o}")
                    nc.sync.dma_start(w[:], w_dram[o * 128:(o + 1) * 128, :])
                    col = ws.tile([128, 1], F32, tag=f"as{tag}{o}")
                    nc.vector.tensor_reduce(
                        col[:], w[:], mybir.AxisListType.X, ALU.add,
                        apply_absolute_value=True,
                    )
                    cols.append(col)
                    wt.append(w)
                # tree-accumulate the per-tile abs sums
                acc = ws.tile([128, 1], F32, tag=f"acc{tag}")
                nc.vector.tensor_tensor(acc[:], cols[0][:], cols[1][:], ALU.add)
                for col in cols[2:]:
                    nc.vector.tensor_tensor(acc[:], acc[:], col[:], ALU.add)
                allsum = ws.tile([128, 1], F32, tag=f"alls{tag}")
                nc.gpsimd.partition_all_reduce(allsum[:], acc[:], 128, ReduceOp.add)
                mc = const.tile([128, 1], F32, tag=f"mc{tag}", name=f"mc{tag}")
                nc.vector.tensor_scalar(
                    out=mc[:], in0=allsum[:], scalar1=1.0 / nelem, scalar2=1e-5,
                    op0=ALU.mult, op1=ALU.max,
                )
                sw = ws.tile([128, 1], F32, tag=f"sw{tag}")
                nc.vector.reciprocal(sw[:], mc[:])
                tern = []
                for o in range(n_tiles):
                    q1 = ws.tile([128, C], F32, tag=f"q1{tag}")
                    nc.scalar.activation(q1[:], wt[o][:], AF.Copy, bias=MAGIC, scale=sw[:])
                    q2 = ws.tile([128, C], F32, tag=f"q2{tag}")
                    nc.vector.tensor_scalar(
                        out=q2[:], in0=q1[:], scalar1=MAGIC, scalar2=1.0,
                        op0=ALU.subtract, op1=ALU.min,
                    )
                    t = ws.tile([128, C], F16, tag=f"t{tag}{o}")
                    nc.vector.tensor_scalar(
                        out=t[:], in0=q2[:], scalar1=-1.0, scalar2=None, op0=ALU.max,
                    )
                    tern.append(t)
                return tern, mc

            t1, mc1 = quant_weights(w1_d, OT, O3 * C, "1")
            t2, mc2 = quant_weights(w2_d, CB, C * C, "2")

            nc.vector.tensor_scalar(out=mc1_128[:], in0=mc1[:], scalar1=1.0 / 128.0,
                                    scalar2=None, op0=ALU.mult)
            nc.vector.tensor_scalar(out=mc1_1024[:], in0=mc1[:], scalar1=1.0 / 1024.0,
                                    scalar2=None, op0=ALU.mult)
            nc.vector.tensor_scalar(out=mc2_128[:], in0=mc2[:], scalar1=1.0 / 128.0,
                                    scalar2=None, op0=ALU.mult)

            # transpose ternary weights:  t1 [o][128, C] -> t1T [c][128, O3]
            for o in range(OT):
                for cb in range(CB):
                    p = ps_tr.tile([128, 128], F16, tag="tr")
                    nc.tensor.transpose(p[:], t1[o][:, cb * 128:(cb + 1) * 128], id16[:])
                    if (o + cb) % 2 == 0:
                        nc.vector.tensor_copy(t1T[cb][:, o * 128:(o + 1) * 128], p[:])
                    else:
                        nc.scalar.copy(t1T[cb][:, o * 128:(o + 1) * 128], p[:])
            for o in range(CB):
                for cb in range(CB):
                    p = ps_tr.tile([128, 128], F16, tag="tr")
                    nc.tensor.transpose(p[:], t2[o][:, cb * 128:(cb + 1) * 128], id16[:])
                    nc.vector.tensor_copy(t2T[cb][:, o * 128:(o + 1) * 128], p[:])

            # biases
            for o in range(6):
                nc.sync.dma_start(
                    b1col[o][:, 0:1],
                    b1_d[o * 128:(o + 1) * 128].rearrange("(p one) -> p one", one=1),
                )
            for o in range(3):
                nc.vector.tensor_scalar(out=bqcol[o][:], in0=b1col[o][:],
                                        scalar1=0.125, scalar2=None, op0=ALU.mult)
            bv_row = ws.tile([1, C], F32, tag="bvr")
            nc.sync.dma_start(bv_row[:], b1_d[2 * C:3 * C].rearrange("(one f) -> one f", one=1))
            nc.gpsimd.partition_broadcast(bv_b[:], bv_row[:])
            b2_row = ws.tile([1, C], F32, tag="b2r")
            nc.sync.dma_start(b2_row[:], b2_d[:].rearrange("(one f) -> one f", one=1))
            nc.gpsimd.partition_broadcast(b2_b[:], b2_row[:])

            # ---------------- x load + act quant ----------------
            amax = const.tile([128, NT], F32)      # clamped per-token absmax
            s_st = const.tile([128, NT], F32)      # 128/amax
            ivv = const.tile([128, NT], F32)       # amax * mc1/128  (v un-scale)
            row_bcast = const.tile([128, N], F32)  # amax broadcast along rows

            mT = [const.tile([128, N], F16, tag=f"mT{c}", name=f"mT{c}") for c in range(CB)]

            with tc.tile_pool(name="xm", bufs=1) as xm:
                # whole-activation tensors: [128, NT, C]; token (j*128+p) = [p, j, :]
                x_all = xm.tile([128, NT, C], F32, tag="x_all")
                nc.sync.dma_start(x_all[:], x_d[:].rearrange("(t p) c -> p t c", p=128))
                nc.vector.tensor_reduce(
                    amax[:], x_all[:], mybir.AxisListType.X, ALU.max,
                    apply_absolute_value=True,
                )
                nc.vector.tensor_scalar(out=amax[:], in0=amax[:],
                                        scalar1=1e-5, scalar2=None, op0=ALU.max)
                nc.vector.reciprocal(s_st[:], amax[:])
                nc.vector.tensor_scalar(out=s_st[:], in0=s_st[:],
                                        scalar1=128.0, scalar2=None, op0=ALU.mult)
                nc.vector.tensor_tensor(ivv[:], amax[:],
                                        mc1_128[:].broadcast_to([128, NT]), ALU.mult)
                # m = clip(round(x * s), -128, 127) : exact vs reference
                nc.vector.tensor_tensor(
                    x_all[:], x_all[:],
                    s_st[:].rearrange("p (t one) -> p t one", one=1).broadcast_to([128, NT, C]),
                    ALU.mult,
                )
                nc.vector.tensor_scalar(out=x_all[:], in0=x_all[:], scalar1=MAGIC,
                                        scalar2=MAGIC, op0=ALU.add, op1=ALU.subtract)
                m_all = xm.tile([128, NT, C], F16, tag="m_all")
                nc.vector.tensor_scalar(out=m_all[:], in0=x_all[:], scalar1=127.0,
                                        scalar2=-128.0, op0=ALU.min, op1=ALU.max)
                # transpose m -> mT  (alternate psum->sbuf copies DVE/ACT)
                for j in range(NT):
                    for cb in range(CB):
                        p = ps_tr.tile([128, 128], F16, tag="tr")
                        nc.tensor.transpose(p[:], m_all[:, j, cb * 128:(cb + 1) * 128], id16[:])
                        if (j * CB + cb) % 2 == 0:
                            nc.vector.tensor_copy(mT[cb][:, j * 128:(j + 1) * 128], p[:])
                        else:
                            nc.scalar.copy(mT[cb][:, j * 128:(j + 1) * 128], p[:])

                # broadcast amax over rows: per chunk, transpose the [128,1]
                # column to a [1,128] row at partition 0, then gpsimd
                # partition-broadcast it across all 128 partitions.
                for j in range(NT):
                    pt = ps_tr.tile([128, 128], F32, tag="tr32")
                    nc.tensor.transpose(pt[0:1, 0:128], amax[:, j:j + 1], id32[:])
                    arow = ws.tile([1, 128], F32, tag="arow", bufs=2)
                    nc.vector.tensor_copy(arow[:], pt[0:1, 0:128])
                    nc.gpsimd.partition_broadcast(
                        row_bcast[:, j * 128:(j + 1) * 128], arow[:]
                    )

        # ---------------- qkv (q/k transposed with dup halves; v natural) ---------
        qdup = [const.tile([128, N], F16, tag=f"qd{h}", name=f"qd{h}") for h in range(H)]
        kdup = [const.tile([128, N], F16, tag=f"kd{h}", name=f"kd{h}") for h in range(H)]
        va = [const.tile([128, H, D + 1], F16, tag=f"va{j}", name=f"va{j}") for j in range(NT)]

        with (
            tc.tile_pool(name="ps_qkv", bufs=2, space="PSUM") as ps_qkv,
            tc.tile_pool(name="ps_v", bufs=2, space="PSUM") as ps_v,
            tc.tile_pool(name="qtmp", bufs=2) as qtmp,
        ):
            for o in range(6):  # 0-2: q, 3-5: k
                mc = mc1_1024 if o < 3 else mc1_128
                bias = bqcol[o] if o < 3 else b1col[o]
                dst = qdup if o < 3 else kdup
                hA, hB = 2 * (o % 3), 2 * (o % 3) + 1
                for half in range(2):
                    base = half * 1024
                    ps = ps_qkv.tile([128, 1024], F32, tag="qkv", name="psqkv")
                    for cb in range(CB):
                        for nq in range(2):
                            nc.tensor.matmul(
                                ps[:, nq * 512:(nq + 1) * 512],
                                t1T[cb][:, o * 128:(o + 1) * 128],
                                mT[cb][:, base + nq * 512:base + (nq + 1) * 512],
                                start=(cb == 0), stop=(cb == CB - 1),
                            )
                    tmp = qtmp.tile([128, 1024], F32, tag="tmp")
                    nc.vector.scalar_tensor_tensor(
                        tmp[:], ps[:], mc[:], row_bcast[:, base:base + 1024],
                        ALU.mult, ALU.mult)
                    # head hA lives in partitions 0:64, hB in 64:128.  Write the
                    # native half with bias, then DMA-duplicate to the other half.
                    nc.vector.tensor_scalar(out=dst[hA][0:64, base:base + 1024],
                                            in0=tmp[0:64, :],
                                            scalar1=bias[0:64, :], scalar2=None,
                                            op0=ALU.add)
                    nc.scalar.activation(dst[hB][64:128, base:base + 1024],
                                         tmp[64:128, :], AF.Identity,
                                         bias=bias[64:128, :], scale=1.0)
                nc.sync.dma_start(dst[hA][64:128, :], dst[hA][0:64, :])
                nc.sync.dma_start(dst[hB][0:64, :], dst[hB][64:128, :])

            # v in natural layout, fused into va ([128, H, D] blocks + ones col)
            for j in range(NT):
                nc.gpsimd.memset(va[j][:], 1.0)
                psv = ps_v.tile([128, C], F32, tag="v")
                for cb in range(CB):
                    nc.tensor.matmul(
                        psv[:], mT[cb][:, j * 128:(j + 1) * 128], t1T[cb][:, 2 * C:3 * C],
                        start=(cb == 0), stop=(cb == CB - 1),
                    )
                nc.vector.scalar_tensor_tensor(
                    va[j][:, 0:H, 0:D],
                    psv[:].rearrange("p (h d) -> p h d", h=H),
                    ivv[:, j:j + 1],
                    bv_b[:].rearrange("p (h d) -> p h d", h=H),
                    ALU.mult, ALU.add,
                )

        # ---------------- attention ----------------
        stage = [const.tile([96, N], F16, tag=f"st{h}", name=f"st{h}") for h in range(H)]
        with (
            tc.tile_pool(name="ps_S", bufs=2, space="PSUM") as ps_S,
            tc.tile_pool(name="ps_O", bufs=1, space="PSUM") as ps_O,
            tc.tile_pool(name="attn", bufs=4) as attn_pool,
        ):
            HN = N // 2  # 1024: S computed in two half-psum tiles, ping-ponged

            def s_half(h, mi, half):
                """S^T[mi-chunk, half*1024:(half+1)*1024]: 2 row-group-paired MMs."""
                ps = ps_S.tile([128, HN], F32, tag="S", name="S")
                ksl = slice(mi * 128, (mi + 1) * 128)
                base = half * HN
                nc.tensor.matmul(ps[:, 0:512], kdup[h][0:64, ksl],
                                 qdup[h][0:64, base:base + 512],
                                 start=True, stop=True, tile_position=(0, 0))
                nc.tensor.matmul(ps[:, 512:1024], kdup[h][64:128, ksl],
                                 qdup[h][64:128, base + 512:base + 1024],
                                 start=True, stop=True, tile_position=(64, 0))
                return ps

            for h in range(H):
                pso = ps_O.tile([65, N], F32, tag="O", name="O")
                shalves = s_half(h, 0, 0), s_half(h, 0, 1)
                for mi in range(NT):
                    ats = []
                    for half in range(2):
                        at = attn_pool.tile([128, HN], F16, tag="at", name="at")
                        nc.scalar.activation(at[:], shalves[half][:], AF.Exp)
                        ats.append(at)
                    # issue next chunk's S before this chunk's AV so the PE can
                    # fill exp latency with S work (S psum slots free as soon as
                    # each exp has read them)
                    if mi + 1 < NT:
                        shalves = s_half(h, mi + 1, 0), s_half(h, mi + 1, 1)
                    for nq in range(4):
                        nc.tensor.matmul(
                            pso[:, nq * 512:(nq + 1) * 512],
                            va[mi][:, h, :],
                            ats[nq // 2][:, (nq % 2) * 512:(nq % 2 + 1) * 512],
                            start=(mi == 0), stop=(mi == NT - 1),
                        )
                nc.vector.tensor_copy(stage[h][0:65, :], pso[:])

        # ---------------- normalize + proj quant + proj ----------------
        with (
            tc.tile_pool(name="ps_F", bufs=2, space="PSUM") as ps_F,
            tc.tile_pool(name="proj", bufs=3) as proj,
        ):
            for j in range(NT):
                onat = proj.tile([128, C], F32, tag="onat")
                for h in range(H):
                    ot = proj.tile([128, 96], F16, tag="ot")
                    nc.sync.dma_start_transpose(ot[:], stage[h][0:96, j * 128:(j + 1) * 128])
                    rs = proj.tile([128, 1], F32, tag="rs")
                    nc.vector.reciprocal(rs[:], ot[:, 64:65])
                    nc.scalar.activation(onat[:, h * D:(h + 1) * D], ot[:, 0:64],
                                         AF.Copy, bias=0.0, scale=rs[:])
                # act quant of onat
                a2 = proj.tile([128, 1], F32, tag="a2")
                nc.vector.tensor_reduce(a2[:], onat[:], mybir.AxisListType.X, ALU.max,
                                        apply_absolute_value=True)
                a2c = proj.tile([128, 1], F32, tag="a2c")
                nc.vector.tensor_scalar(out=a2c[:], in0=a2[:], scalar1=1e-5,
                                        scalar2=None, op0=ALU.max)
                s2 = proj.tile([128, 1], F32, tag="s2")
                nc.vector.reciprocal(s2[:], a2c[:])
                nc.vector.tensor_scalar(out=s2[:], in0=s2[:], scalar1=128.0,
                                        scalar2=None, op0=ALU.mult)
                iv2 = proj.tile([128, 1], F32, tag="iv2")
                nc.vector.tensor_tensor(iv2[:], a2c[:], mc2_128[:], ALU.mult)
                mq1 = proj.tile([128, C], F32, tag="pq1")
                nc.scalar.activation(mq1[:], onat[:], AF.Copy, bias=MAGIC, scale=s2[:])
                mq2 = proj.tile([128, C], F32, tag="pq2")
                nc.vector.tensor_scalar(out=mq2[:], in0=mq1[:], scalar1=MAGIC,
                                        scalar2=127.0, op0=ALU.subtract, op1=ALU.min)
                m2 = proj.tile([128, C], F16, tag="m2")
                nc.vector.tensor_scalar(out=m2[:], in0=mq2[:], scalar1=-128.0,
                                        scalar2=None, op0=ALU.max)
                # transpose m2 tile -> lhsT pieces, then proj matmul
                psf = ps_F.tile([128, C], F32, tag="F")
                for cb in range(CB):
                    piece = proj.tile([128, 128], F16, tag="piece")
                    nc.sync.dma_start_transpose(piece[:], m2[:, cb * 128:(cb + 1) * 128])
                    nc.tensor.matmul(psf[:], piece[:], t2T[cb][:, 0:C],
                                     start=(cb == 0), stop=(cb == CB - 1))
                yt = proj.tile([128, C], F32, tag="y")
                nc.vector.scalar_tensor_tensor(yt[:], psf[:], iv2[:], b2_b[:],
                                               ALU.mult, ALU.add)
                nc.sync.dma_start(y_d[j * 128:(j + 1) * 128, :], yt[:])


_CACHE = {}


def _get_program():
    if "nc" not in _CACHE:
        _CACHE["nc"] = build_program()
    return _CACHE["nc"]


def kernel(x, qkv_w, qkv_b, proj_w, proj_b):
    x = np.ascontiguousarray(np.asarray(x, dtype=np.float32))
    qkv_w = np.ascontiguousarray(np.asarray(qkv_w, dtype=np.float32))
    qkv_b = np.ascontiguousarray(np.asarray(qkv_b, dtype=np.float32))
    proj_w = np.ascontiguousarray(np.asarray(proj_w, dtype=np.float32))
    proj_b = np.ascontiguousarray(np.asarray(proj_b, dtype=np.float32))

    nc = _get_program()
    in_maps = [
        {"x": x[b], "qkv_w": qkv_w, "qkv_b": qkv_b, "proj_w": proj_w, "proj_b": proj_b}
        for b in range(B)
    ]
    res = bass_utils.run_bass_kernel_spmd(nc, in_maps, core_ids=list(range(B)))
    out = np.stack([res.results[b]["y"] for b in range(B)], axis=0)
    _CACHE["last_results"] = res
    return out


# revision 13
# speedup vs baseline: 1.3333x; 1.3333x over previous
"""Trainium2 Bass kernel for quantized (BitNet-style) multi-head attention.

Model (per batch element): bitlinear(qkv) -> 6-head softmax attention -> bitlinear(proj)
with B=8, N=2048, C=384, H=6, D=64.

Sharding: pure data parallel — one batch element per NeuronCore (8 cores),
weights replicated, no collectives.

Device algorithm highlights:
  * act/weight quantization reproduced bit-exactly (exact 128/amax divide +
    magic-number round-to-nearest-even), so the integer matmuls run on the PE
    in fp16 exactly (|ints| <= 128, ternary weights; fp32 PSUM accumulate).
  * attention computed in S^T layout: S^T[m,n] = sum_d k^T[d,m] q^T[d,n].
    D=64 -> two concurrent row-tiled matmuls (tile_position (0,0)/(64,0))
    using duplicated q/k partition halves to keep the full PE array busy.
  * softmax skips the max-subtraction (scores are bounded ~|0.5|) and gets the
    row-sums for free from an appended ones-column in V during attn @ V.
  * exp runs on ScalarE straight out of PSUM at N=2048 per instruction.
"""

import sys
import os

sys.path.insert(0, "/opt/trn_rl_repo")

import numpy as np

import concourse.bass as bass
import concourse.mybir as mybir
import concourse.tile as tile
import concourse.bacc as bacc
from concourse import bass_utils
from concourse.bass_isa import ReduceOp
from concourse.masks import make_identity

F32 = mybir.dt.float32
F16 = mybir.dt.float16
AF = mybir.ActivationFunctionType
ALU = mybir.AluOpType

B, N, C = 8, 2048, 384
H, D = 6, 64
O3 = 3 * C  # 1152
NT = N // 128   # 16 token tiles
CB = C // 128   # 3 contraction blocks
OT = O3 // 128  # 9 qkv output row tiles
MAGIC = float(1.5 * 2**23)  # fp32 round-to-nearest-even via add/sub


def build_program():
    nc = bacc.Bacc("TRN2", target_bir_lowering=False, debug=False, num_devices=8)

    x_d = nc.dram_tensor("x", [N, C], F32, kind="ExternalInput")
    w1_d = nc.dram_tensor("qkv_w", [O3, C], F32, kind="ExternalInput")
    b1_d = nc.dram_tensor("qkv_b", [O3], F32, kind="ExternalInput")
    w2_d = nc.dram_tensor("proj_w", [C, C], F32, kind="ExternalInput")
    b2_d = nc.dram_tensor("proj_b", [C], F32, kind="ExternalInput")
    y_d = nc.dram_tensor("y", [N, C], F32, kind="ExternalOutput")

    with tile.TileContext(nc) as tc:
        _body(nc, tc, x_d, w1_d, b1_d, w2_d, b2_d, y_d)
    nc.compile()
    return nc


def _body(nc, tc, x_d, w1_d, b1_d, w2_d, b2_d, y_d):
    from contextlib import ExitStack

    ctx = ExitStack()
    with ctx:
        const = ctx.enter_context(tc.tile_pool(name="const", bufs=1))
        # identities for PE transposes
        id16 = const.tile([128, 128], F16)
        make_identity(nc, id16[:])
        id32 = const.tile([128, 128], F32)
        make_identity(nc, id32[:])
        c1 = const.tile([128, 1], F32)
        nc.gpsimd.memset(c1[:], 1.0)
        c128 = const.tile([128, 1], F32)
        nc.gpsimd.memset(c128[:], 128.0)

        # ---------------- weights preamble ----------------
        t1T = [const.tile([128, O3], F16, tag=f"t1T{c}", name=f"t1T{c}") for c in range(CB)]
        t2T = [const.tile([128, C], F16, tag=f"t2T{c}", name=f"t2T{c}") for c in range(CB)]
        b1col = [const.tile([128, 1], F32, tag=f"b1c{o}", name=f"b1c{o}") for o in range(6)]
        bqcol = [const.tile([128, 1], F32, tag=f"bqc{o}", name=f"bqc{o}") for o in range(3)]
        bv_b = const.tile([128, C], F32)
        b2_b = const.tile([128, C], F32)
        mc1_128 = const.tile([128, 1], F32)
        mc1_1024 = const.tile([128, 1], F32)
        mc2_128 = const.tile([128, 1], F32)

        with (
            tc.tile_pool(name="wstage", bufs=1) as ws,
            tc.tile_pool(name="ps_tr", bufs=2, space="PSUM") as ps_tr,
        ):
            def quant_weights(w_dram, n_tiles, nelem, tag):
                """Load fp32 weight tiles, return (list of fp16 ternary tiles, mc tile)
                where mc = max(mean|w|, 1e-5) broadcast to [128,1] (equals 1/scale)."""
                wt = []
                cols = []
                for o in range(n_tiles):
                    w = ws.tile([128, C], F32, tag=f"w{tag}{o}")
                    nc.sync.dma_start(w[:], w_dram[o * 128:(o + 1) * 128, :])
                    col = ws.tile([128, 1], F32, tag=f"as{tag}{o}")
                    nc.vector.tensor_reduce(
                        col[:], w[:], mybir.AxisListType.X, ALU.add,
                        apply_absolute_value=True,
                    )
                    cols.append(col)
                    wt.append(w)
                # tree-accumulate the per-tile abs sums
                acc = ws.tile([128, 1], F32, tag=f"acc{tag}")
                nc.vector.tensor_tensor(acc[:], cols[0][:], cols[1][:], ALU.add)
                for col in cols[2:]:
                    nc.vector.tensor_tensor(acc[:], acc[:], col[:], ALU.add)
                allsum = ws.tile([128, 1], F32, tag=f"alls{tag}")
                nc.gpsimd.partition_all_reduce(allsum[:], acc[:], 128, ReduceOp.add)
                mc = const.tile([128, 1], F32, tag=f"mc{tag}", name=f"mc{tag}")
                nc.vector.tensor_scalar(
                    out=mc[:], in0=allsum[:], scalar1=1.0 / nelem, scalar2=1e-5,
                    op0=ALU.mult, op1=ALU.max,
                )
                sw = ws.tile([128, 1], F32, tag=f"sw{tag}")
                nc.vector.reciprocal(sw[:], mc[:])
                tern = []
                for o in range(n_tiles):
                    q1 = ws.tile([128, C], F32, tag=f"q1{tag}")
                    nc.scalar.activation(q1[:], wt[o][:], AF.Copy, bias=MAGIC, scale=sw[:])
                    q2 = ws.tile([128, C], F32, tag=f"q2{tag}")
                    nc.vector.tensor_scalar(
                        out=q2[:], in0=q1[:], scalar1=MAGIC, scalar2=1.0,
                        op0=ALU.subtract, op1=ALU.min,
                    )
                    t = ws.tile([128, C], F16, tag=f"t{tag}{o}")
                    nc.vector.tensor_scalar(
                        out=t[:], in0=q2[:], scalar1=-1.0, scalar2=None, op0=ALU.max,
                    )
                    tern.append(t)
                return tern, mc

            t1, mc1 = quant_weights(w1_d, OT, O3 * C, "1")
            t2, mc2 = quant_weights(w2_d, CB, C * C, "2")

            nc.vector.tensor_scalar(out=mc1_128[:], in0=mc1[:], scalar1=1.0 / 128.0,
                                    scalar2=None, op0=ALU.mult)
            nc.vector.tensor_scalar(out=mc1_1024[:], in0=mc1[:], scalar1=1.0 / 1024.0,
                                    scalar2=None, op0=ALU.mult)
            nc.vector.tensor_scalar(out=mc2_128[:], in0=mc2[:], scalar1=1.0 / 128.0,
                                    scalar2=None, op0=ALU.mult)

            # transpose ternary weights:  t1 [o][128, C] -> t1T [c][128, O3]
            for o in range(OT):
                for cb in range(CB):
                    p = ps_tr.tile([128, 128], F16, tag="tr")
                    nc.tensor.transpose(p[:], t1[o][:, cb * 128:(cb + 1) * 128], id16[:])
                    if (o + cb) % 2 == 0:
                        nc.vector.tensor_copy(t1T[cb][:, o * 128:(o + 1) * 128], p[:])
                    else:
                        nc.scalar.copy(t1T[cb][:, o * 128:(o + 1) * 128], p[:])
            for o in range(CB):
                for cb in range(CB):
                    p = ps_tr.tile([128, 128], F16, tag="tr")
                    nc.tensor.transpose(p[:], t2[o][:, cb * 128:(cb + 1) * 128], id16[:])
                    nc.vector.tensor_copy(t2T[cb][:, o * 128:(o + 1) * 128], p[:])

            # biases
            for o in range(6):
                nc.sync.dma_start(
                    b1col[o][:, 0:1],
                    b1_d[o * 128:(o + 1) * 128].rearrange("(p one) -> p one", one=1),
                )
            for o in range(3):
                nc.vector.tensor_scalar(out=bqcol[o][:], in0=b1col[o][:],
                                        scalar1=0.125, scalar2=None, op0=ALU.mult)
            bv_row = ws.tile([1, C], F32, tag="bvr")
            nc.sync.dma_start(bv_row[:], b1_d[2 * C:3 * C].rearrange("(one f) -> one f", one=1))
            nc.gpsimd.partition_broadcast(bv_b[:], bv_row[:])
            b2_row = ws.tile([1, C], F32, tag="b2r")
            nc.sync.dma_start(b2_row[:], b2_d[:].rearrange("(one f) -> one f", one=1))
            nc.gpsimd.partition_broadcast(b2_b[:], b2_row[:])

            # ---------------- x load + act quant ----------------
            amax = const.tile([128, NT], F32)      # clamped per-token absmax
            s_st = const.tile([128, NT], F32)      # 128/amax
            ivv = const.tile([128, NT], F32)       # amax * mc1/128  (v un-scale)
            row_bcast = const.tile([128, N], F32)  # amax broadcast along rows

            mT = [const.tile([128, N], F16, tag=f"mT{c}", name=f"mT{c}") for c in range(CB)]

            with tc.tile_pool(name="xm", bufs=1) as xm:
                # whole-activation tensors: [128, NT, C]; token (j*128+p) = [p, j, :]
                x_all = xm.tile([128, NT, C], F32, tag="x_all")
                nc.sync.dma_start(x_all[:], x_d[:].rearrange("(t p) c -> p t c", p=128))
                nc.vector.tensor_reduce(
                    amax[:], x_all[:], mybir.AxisListType.X, ALU.max,
                    apply_absolute_value=True,
                )
                nc.vector.tensor_scalar(out=amax[:], in0=amax[:],
                                        scalar1=1e-5, scalar2=None, op0=ALU.max)
                nc.vector.reciprocal(s_st[:], amax[:])
                nc.vector.tensor_scalar(out=s_st[:], in0=s_st[:],
                                        scalar1=128.0, scalar2=None, op0=ALU.mult)
                nc.vector.tensor_tensor(ivv[:], amax[:],
                                        mc1_128[:].broadcast_to([128, NT]), ALU.mult)
                # m = clip(round(x * s), -128, 127) : exact vs reference
                nc.vector.tensor_tensor(
                    x_all[:], x_all[:],
                    s_st[:].rearrange("p (t one) -> p t one", one=1).broadcast_to([128, NT, C]),
                    ALU.mult,
                )
                nc.vector.tensor_scalar(out=x_all[:], in0=x_all[:], scalar1=MAGIC,
                                        scalar2=MAGIC, op0=ALU.add, op1=ALU.subtract)
                m_all = xm.tile([128, NT, C], F16, tag="m_all")
                nc.vector.tensor_scalar(out=m_all[:], in0=x_all[:], scalar1=127.0,
                                        scalar2=-128.0, op0=ALU.min, op1=ALU.max)
                # transpose m -> mT  (alternate psum->sbuf copies DVE/ACT)
                for j in range(NT):
                    for cb in range(CB):
                        p = ps_tr.tile([128, 128], F16, tag="tr")
                        nc.tensor.transpose(p[:], m_all[:, j, cb * 128:(cb + 1) * 128], id16[:])
                        if (j * CB + cb) % 2 == 0:
                            nc.vector.tensor_copy(mT[cb][:, j * 128:(j + 1) * 128], p[:])
                        else:
                            nc.scalar.copy(mT[cb][:, j * 128:(j + 1) * 128], p[:])

                # broadcast amax over rows: per chunk, transpose the [128,1]
                # column to a [1,128] row at partition 0, then gpsimd
                # partition-broadcast it across all 128 partitions.
                for j in range(NT):
                    pt = ps_tr.tile([128, 128], F32, tag="tr32")
                    nc.tensor.transpose(pt[0:1, 0:128], amax[:, j:j + 1], id32[:])
                    arow = ws.tile([1, 128], F32, tag="arow", bufs=2)
                    nc.vector.tensor_copy(arow[:], pt[0:1, 0:128])
                    nc.gpsimd.partition_broadcast(
                        row_bcast[:, j * 128:(j + 1) * 128], arow[:]
                    )

        # ---------------- qkv (q/k transposed with dup halves; v natural) ---------
        qdup = [const.tile([128, N], F16, tag=f"qd{h}", name=f"qd{h}") for h in range(H)]
        kdup = [const.tile([128, N], F16, tag=f"kd{h}", name=f"kd{h}") for h in range(H)]
        va = [const.tile([128, H, D + 1], F16, tag=f"va{j}", name=f"va{j}") for j in range(NT)]

        with (
            tc.tile_pool(name="ps_qkv", bufs=2, space="PSUM") as ps_qkv,
            tc.tile_pool(name="ps_v", bufs=2, space="PSUM") as ps_v,
            tc.tile_pool(name="qtmp", bufs=2) as qtmp,
        ):
            for o in range(6):  # 0-2: q, 3-5: k
                mc = mc1_1024 if o < 3 else mc1_128
                bias = bqcol[o] if o < 3 else b1col[o]
                dst = qdup if o < 3 else kdup
                hA, hB = 2 * (o % 3), 2 * (o % 3) + 1
                for half in range(2):
                    base = half * 1024
                    ps = ps_qkv.tile([128, 1024], F32, tag="qkv", name="psqkv")
                    for cb in range(CB):
                        for nq in range(2):
                            nc.tensor.matmul(
                                ps[:, nq * 512:(nq + 1) * 512],
                                t1T[cb][:, o * 128:(o + 1) * 128],
                                mT[cb][:, base + nq * 512:base + (nq + 1) * 512],
                                start=(cb == 0), stop=(cb == CB - 1),
                            )
                    tmp = qtmp.tile([128, 1024], F32, tag="tmp")
                    nc.vector.scalar_tensor_tensor(
                        tmp[:], ps[:], mc[:], row_bcast[:, base:base + 1024],
                        ALU.mult, ALU.mult)
                    # head hA lives in partitions 0:64, hB in 64:128.  Write the
                    # native half with bias, then DMA-duplicate to the other half.
                    nc.vector.tensor_scalar(out=dst[hA][0:64, base:base + 1024],
                                            in0=tmp[0:64, :],
                                            scalar1=bias[0:64, :], scalar2=None,
                                            op0=ALU.add)
                    nc.scalar.activation(dst[hB][64:128, base:base + 1024],
                                         tmp[64:128, :], AF.Identity,
                                         bias=bias[64:128, :], scale=1.0)
                nc.sync.dma_start(dst[hA][64:128, :], dst[hA][0:64, :])
                nc.sync.dma_start(dst[hB][0:64, :], dst[hB][64:128, :])

            # v in natural layout, fused into va ([128, H, D] blocks + ones col)
            for j in range(NT):
                nc.gpsimd.memset(va[j][:], 1.0)
                psv = ps_v.tile([128, C], F32, tag="v")
                for cb in range(CB):
                    nc.tensor.matmul(
                        psv[:], mT[cb][:, j * 128:(j + 1) * 128], t1T[cb][:, 2 * C:3 * C],
                        start=(cb == 0), stop=(cb == CB - 1),
                    )
                nc.vector.scalar_tensor_tensor(
                    va[j][:, 0:H, 0:D],
                    psv[:].rearrange("p (h d) -> p h d", h=H),
                    ivv[:, j:j + 1],
                    bv_b[:].rearrange("p (h d) -> p h d", h=H),
                    ALU.mult, ALU.add,
                )

        # ---------------- attention ----------------
        stage = [const.tile([96, N], F16, tag=f"st{h}", name=f"st{h}") for h in range(H)]
        with (
            tc.tile_pool(name="ps_S", bufs=2, space="PSUM") as ps_S,
            tc.tile_pool(name="ps_O", bufs=1, space="PSUM") as ps_O,
            tc.tile_pool(name="attn", bufs=4) as attn_pool,
        ):
            HN = N // 2  # 1024: S computed in two half-psum tiles, ping-ponged

            def s_half(h, mi, half):
                """S^T[mi-chunk, half*1024:(half+1)*1024]: 2 row-group-paired MMs."""
                ps = ps_S.tile([128, HN], F32, tag="S", name="S")
                ksl = slice(mi * 128, (mi + 1) * 128)
                base = half * HN
                nc.tensor.matmul(ps[:, 0:512], kdup[h][0:64, ksl],
                                 qdup[h][0:64, base:base + 512],
                                 start=True, stop=True, tile_position=(0, 0))
                nc.tensor.matmul(ps[:, 512:1024], kdup[h][64:128, ksl],
                                 qdup[h][64:128, base + 512:base + 1024],
                                 start=True, stop=True, tile_position=(64, 0))
                return ps

            for h in range(H):
                pso = ps_O.tile([65, N], F32, tag="O", name="O")
                shalves = s_half(h, 0, 0), s_half(h, 0, 1)
                for mi in range(NT):
                    ats = []
                    for half in range(2):
                        at = attn_pool.tile([128, HN], F16, tag="at", name="at")
                        nc.scalar.activation(at[:], shalves[half][:], AF.Exp)
                        ats.append(at)
                    # issue next chunk's S before this chunk's AV so the PE can
                    # fill exp latency with S work (S psum slots free as soon as
                    # each exp has read them)
                    if mi + 1 < NT:
                        shalves = s_half(h, mi + 1, 0), s_half(h, mi + 1, 1)
                    for nq in range(4):
                        nc.tensor.matmul(
                            pso[:, nq * 512:(nq + 1) * 512],
                            va[mi][:, h, :],
                            ats[nq // 2][:, (nq % 2) * 512:(nq % 2 + 1) * 512],
                            start=(mi == 0), stop=(mi == NT - 1),
                        )
                nc.vector.tensor_copy(stage[h][0:65, :], pso[:])

        # ---------------- normalize + proj quant + proj ----------------
        with (
            tc.tile_pool(name="ps_tr2", bufs=4, space="PSUM") as ps_tr2,
            tc.tile_pool(name="ps_F", bufs=2, space="PSUM") as ps_F,
            tc.tile_pool(name="proj", bufs=3) as proj,
        ):
            for j in range(NT):
                onat = proj.tile([128, C], F32, tag="onat")
                for h in range(H):
                    p = ps_tr2.tile([128, 128], F16, tag="tr2h")
                    nc.tensor.transpose(
                        p[0:128, 0:65], stage[h][0:65, j * 128:(j + 1) * 128],
                        id16[0:65, 0:65]
                    )
                    rs = proj.tile([128, 1], F32, tag="rs")
                    nc.vector.reciprocal(rs[:], p[:, 64:65])
                    nc.scalar.activation(onat[:, h * D:(h + 1) * D], p[:, 0:64],
                                         AF.Copy, bias=0.0, scale=rs[:])
                # act quant of onat
                a2 = proj.tile([128, 1], F32, tag="a2")
                nc.vector.tensor_reduce(a2[:], onat[:], mybir.AxisListType.X, ALU.max,
                                        apply_absolute_value=True)
                a2c = proj.tile([128, 1], F32, tag="a2c")
                nc.vector.tensor_scalar(out=a2c[:], in0=a2[:], scalar1=1e-5,
                                        scalar2=None, op0=ALU.max)
                s2 = proj.tile([128, 1], F32, tag="s2")
                nc.vector.reciprocal(s2[:], a2c[:])
                nc.vector.tensor_scalar(out=s2[:], in0=s2[:], scalar1=128.0,
                                        scalar2=None, op0=ALU.mult)
                iv2 = proj.tile([128, 1], F32, tag="iv2")
                nc.vector.tensor_tensor(iv2[:], a2c[:], mc2_128[:], ALU.mult)
                mq1 = proj.tile([128, C], F32, tag="pq1")
                nc.scalar.activation(mq1[:], onat[:], AF.Copy, bias=MAGIC, scale=s2[:])
                mq2 = proj.tile([128, C], F32, tag="pq2")
                nc.vector.tensor_scalar(out=mq2[:], in0=mq1[:], scalar1=MAGIC,
                                        scalar2=127.0, op0=ALU.subtract, op1=ALU.min)
                m2 = proj.tile([128, C], F16, tag="m2")
                nc.vector.tensor_scalar(out=m2[:], in0=mq2[:], scalar1=-128.0,
                                        scalar2=None, op0=ALU.max)
                # transpose m2 tile -> lhsT pieces, then proj matmul
                psf = ps_F.tile([128, C], F32, tag="F")
                for cb in range(CB):
                    p = ps_tr2.tile([128, 128], F16, tag="tr2h")
                    nc.tensor.transpose(p[:], m2[:, cb * 128:(cb + 1) * 128], id16[:])
                    piece = proj.tile([128, 128], F16, tag="piece")
                    nc.vector.tensor_copy(piece[:], p[:])
                    nc.tensor.matmul(psf[:], piece[:], t2T[cb][:, 0:C],
                                     start=(cb == 0), stop=(cb == CB - 1))
                yt = proj.tile([128, C], F32, tag="y")
                nc.vector.scalar_tensor_tensor(yt[:], psf[:], iv2[:], b2_b[:],
                                               ALU.mult, ALU.add)
                nc.sync.dma_start(y_d[j * 128:(j + 1) * 128, :], yt[:])


_CACHE = {}


def _get_program():
    if "nc" not in _CACHE:
        _CACHE["nc"] = build_program()
    return _CACHE["nc"]


def kernel(x, qkv_w, qkv_b, proj_w, proj_b):
    x = np.ascontiguousarray(np.asarray(x, dtype=np.float32))
    qkv_w = np.ascontiguousarray(np.asarray(qkv_w, dtype=np.float32))
    qkv_b = np.ascontiguousarray(np.asarray(qkv_b, dtype=np.float32))
    proj_w = np.ascontiguousarray(np.asarray(proj_w, dtype=np.float32))
    proj_b = np.ascontiguousarray(np.asarray(proj_b, dtype=np.float32))

    nc = _get_program()
    in_maps = [
        {"x": x[b], "qkv_w": qkv_w, "qkv_b": qkv_b, "proj_w": proj_w, "proj_b": proj_b}
        for b in range(B)
    ]
    res = bass_utils.run_bass_kernel_spmd(nc, in_maps, core_ids=list(range(B)))
    out = np.stack([res.results[b]["y"] for b in range(B)], axis=0)
    _CACHE["last_results"] = res
    return out
